# revision 20
# baseline (speedup 1.0000x reference)
"""Trainium2 Bass kernel for BlockGlobalSelfAttention (8-core SPMD).

Sharding: core c handles batch n = c//4, head group hg = c%4 (4 heads each).
Each core is fully independent (no collectives).

Algorithm reformulation (validated against the jax reference to ~2e-6):
  - top-16-per-64-block selection via 16th-largest threshold (max8 x2 +
    match_replace), giving 0/1 masks is_g / is_l per token
  - both attention passes share one set of exp(q.k/8) score tiles S^T
    [128 keys x 384 queries] (keys on partitions); selection masks are
    applied multiplicatively on the V side (Vl = V*is_l, Vg = V*is_g,
    with an extra ones*mask column to accumulate denominators)
  - merged output = (ctxU_l + ctxU_g + E_bos*v0) / (Z_l + Z_g + E_bos):
    algebraically identical to the reference's logsumexp-sigmoid merge
  - BOS key (token 0) is an always-attended rank-1 update (matches the
    reference's gk double-count); query row 0 replaced by an unscaled
    full-attention pass
attention_mask is all-zeros per the problem spec and is ignored.
"""
import sys, os
sys.path.insert(0, "/opt/trn_rl_repo")
import numpy as np

N, T, HID, NH, D = 2, 4096, 1024, 16, 64
NB = 64          # 64-token selection blocks per head
NCH = 32         # 128-query chunks
FPC = 256        # features per core (4 heads)
NPAIR = 2        # head pairs per core
W_DT_NP = np.float32

_CACHE = {"nc": None}


def emit(nc, tc, ctx, hidT, hidTb, wqT, wkT, wkTb, wvT, bias, bvrow, bkqrow, O):
    import concourse.mybir as mybir
    from concourse.masks import make_identity
    dt = mybir.dt.float32
    bf = mybir.dt.bfloat16
    AF = mybir.ActivationFunctionType
    ALU = mybir.AluOpType

    const = ctx.enter_context(tc.tile_pool(name="const", bufs=1))
    big = ctx.enter_context(tc.tile_pool(name="big", bufs=1))
    hidp = ctx.enter_context(tc.tile_pool(name="hidp", bufs=9))
    wp = ctx.enter_context(tc.tile_pool(name="wp", bufs=1))
    evp = ctx.enter_context(tc.tile_pool(name="evp", bufs=1))
    ps1 = ctx.enter_context(tc.tile_pool(name="ps1", bufs=2, space="PSUM"))
    sel = ctx.enter_context(tc.tile_pool(name="sel", bufs=2))
    ep = ctx.enter_context(tc.tile_pool(name="ep", bufs=8))
    epps = ctx.enter_context(tc.tile_pool(name="epps", bufs=2, space="PSUM"))
    ctxp = ctx.enter_context(tc.tile_pool(name="ctxp", bufs=1, space="PSUM"))
    r0p = ctx.enter_context(tc.tile_pool(name="r0p", bufs=1, space="PSUM"))
    outp = ctx.enter_context(tc.tile_pool(name="outp", bufs=3))

    # ---- constants ----
    bias_sb = const.tile([128, 6], dt)
    nc.sync.dma_start(bias_sb[:, :], bias.ap()[:, :])
    bv_sb = const.tile([1, 256], bf)
    nc.sync.dma_start(bv_sb[:, :], bvrow.ap()[:, :])
    ones_row = const.tile([1, 128], bf)
    nc.vector.memset(ones_row[:, :], 1.0)
    ones_rowf = const.tile([1, 128], dt)
    nc.vector.memset(ones_rowf[:, :], 1.0)
    bkq_sb = const.tile([1, 256], dt)
    nc.sync.dma_start(bkq_sb[:, :], bkqrow.ap()[:, :])
    ident = const.tile([128, 128], dt)
    make_identity(nc, ident[:, :])

    # ---- weights ----
    w_sb = {}
    for name, wt, wdt in (("q", wqT, bf), ("k", wkT, dt), ("kb", wkTb, bf), ("v", wvT, bf)):
        w = wp.tile([128, 8 * 256], wdt, tag=f"w{name}", name=f"w{name}")
        for c in range(8):
            nc.sync.dma_start(
                w[:, c * 256:(c + 1) * 256],
                wt.ap()[c * 128:(c + 1) * 128, :])
        w_sb[name] = w

    # ---- persistent big buffers ----
    qT = [big.tile([128, T], bf, tag=f"qT{p}", name=f"qT{p}") for p in range(NPAIR)]
    kTb = [big.tile([128, T], bf, tag=f"kTb{p}", name=f"kTb{p}") for p in range(NPAIR)]
    kTlo = [big.tile([128, T], bf, tag=f"kTlo{p}", name=f"kTlo{p}") for p in range(NPAIR)]
    nrm_all = big.tile([128, 128], dt, tag="nrm_all", name="nrm_all")
    vext = big.tile([128, 32 * 260], bf, tag="vext", name="vext")
    vl = big.tile([128, 32 * 260], bf, tag="vl", name="vl")
    vg = big.tile([128, 32 * 260], bf, tag="vg", name="vg")
    ebos = [big.tile([33, T], bf, tag=f"ebos{p}", name=f"ebos{p}") for p in range(NPAIR)]
    e0all = big.tile([128, 128], bf, tag="e0all", name="e0all")
    q0f = [big.tile([128, 1], dt, tag=f"q0f{p}", name=f"q0f{p}") for p in range(NPAIR)]
    q0s = [big.tile([128, 2], bf, tag=f"q0s{p}", name=f"q0s{p}") for p in range(NPAIR)]
    zq0 = [big.tile([128, 1], bf, tag=f"zq0{p}", name=f"zq0{p}") for p in range(NPAIR)]
    masks_g = [big.tile([128, 32], dt, tag=f"mg{h}", name=f"mg{h}") for h in range(4)]
    masks_l = [big.tile([128, 32], dt, tag=f"ml{h}", name=f"ml{h}") for h in range(4)]
    v0st = [big.tile([33, 65], bf, tag=f"v0st{p}", name=f"v0st{p}") for p in range(NPAIR)]
    r0 = const.tile([1, 4], dt)
    row0_sb = const.tile([1, 256], dt)

    # ================= Phase B: projections =================
    for tch in range(8):
        t0 = tch * 512
        hid = [hidp.tile([128, 512], dt, tag="hid", name=f"hid{tch}_{i}", bufs=9)
               for i in range(8)]
        hidb = [hidp.tile([128, 512], bf, tag="hidb", name=f"hidb{tch}_{i}", bufs=9)
                for i in range(8)]
        for c in range(8):
            nc.sync.dma_start(hid[c][:, :], hidT.ap()[c * 128:(c + 1) * 128, t0:t0 + 512])
            nc.sync.dma_start(hidb[c][:, :], hidTb.ap()[c * 128:(c + 1) * 128, t0:t0 + 512])
        # k^T bf16 (attention); q^T bf16
        for p in range(NPAIR):
            ps = ps1.tile([128, 512], dt, tag="t1", name="kps")
            for c in range(8):
                nc.tensor.matmul(
                    ps[:, :],
                    w_sb["kb"][:, c * 256 + p * 128: c * 256 + (p + 1) * 128],
                    hidb[c][:, :], start=(c == 0), stop=(c == 7))
            nc.scalar.activation(kTb[p][:, t0:t0 + 512], ps[:, :],
                                 AF.Identity, bias=bias_sb[:, 2 + p:3 + p])
            nc.vector.scalar_tensor_tensor(
                kTlo[p][:, t0:t0 + 512], ps[:, :], bias_sb[:, 2 + p:3 + p],
                kTb[p][:, t0:t0 + 512], op0=ALU.add, op1=ALU.subtract)
        for p in range(NPAIR):
            ps = ps1.tile([128, 512], dt, tag="t1", name="qps")
            for c in range(8):
                nc.tensor.matmul(
                    ps[:, :],
                    w_sb["q"][:, c * 256 + p * 128: c * 256 + (p + 1) * 128],
                    hidb[c][:, :], start=(c == 0), stop=(c == 7))
            nc.scalar.activation(qT[p][:, t0:t0 + 512], ps[:, :],
                                 AF.Identity, bias=bias_sb[:, p:p + 1])
            if tch == 0:
                nc.vector.memset(zq0[p][:, :], 0.0)
                nc.vector.tensor_scalar_add(q0f[p][:, :], ps[:, 0:1],
                                            bias_sb[:, p:p + 1])
                nc.vector.tensor_copy(q0s[p][:, 0:1], q0f[p][:, :])
                nc.vector.tensor_tensor(q0s[p][:, 1:2], q0f[p][:, :], q0s[p][:, 0:1],
                                        op=ALU.subtract)
        # V natural (tokens on partitions), strided into [V|1] groups of 65
        for st in range(4):
            tt = tch * 4 + st
            ps = ps1.tile([128, 512], dt, tag="t1", name="vps")[:, 0:260]
            psv = ps[:, :].rearrange("p (h d) -> p h d", h=4)[:, :, 0:64]
            for c in range(8):
                nc.tensor.matmul(psv, hidb[c][:, st * 128:(st + 1) * 128],
                                 w_sb["v"][:, c * 256:(c + 1) * 256],
                                 start=(c == 0), stop=False)
            nc.tensor.matmul(psv, ones_row[:, 0:128],
                             bv_sb[:, :], start=False, stop=True)
            nc.vector.tensor_copy(vext[:, tt * 260:(tt + 1) * 260], ps[:, :])
            vv = vext[:, tt * 260:(tt + 1) * 260].rearrange("p (h d) -> p h d", h=4)
            nc.vector.memset(vv[:, :, 64:65], 1.0)
        # k natural (fp32, PSUM-only) -> squares -> per-head norms [128 tok, 4]
        for st in range(4):
            tt = tch * 4 + st
            kn = ps1.tile([128, 512], dt, tag="t1", name="kn")[:, 0:256]
            for c in range(8):
                nc.tensor.matmul(kn[:, :], hid[c][:, st * 128:(st + 1) * 128],
                                 w_sb["k"][:, c * 256:(c + 1) * 256],
                                 start=(c == 0), stop=False)
            nc.tensor.matmul(kn[:, :], ones_rowf[:, 0:128],
                             bkq_sb[:, :], start=False, stop=True)
            sqn = evp.tile([128, 256], dt, tag="sqn", name=f"sqn{tt}", bufs=3)
            nc.scalar.activation(sqn[:, :], kn[:, :], AF.Square)
            nc.vector.tensor_reduce(
                nrm_all[:, tt * 4:(tt + 1) * 4],
                sqn[:, :].rearrange("p (h d) -> p h d", h=4),
                axis=mybir.AxisListType.X, op=ALU.add)

    # ================= Phase C: selection =================
    for h in range(4):
        ntp = ps1.tile([128, 512], dt, tag="t1", name="ntp")[0:32, 0:128]
        nc.tensor.transpose(ntp[:, :], nrm_all[:, h:128:4], ident[:, :])
        nrmT = sel.tile([32, 128], dt, tag="nrmT")
        nc.vector.tensor_copy(nrmT[:, :], ntp[:, :])
        scr = sel.tile([32, 128], dt, tag="scr")
        isg2 = sel.tile([32, 128], dt, tag="isg2")
        for s in range(2):
            cs = slice(s * 64, s * 64 + 64)
            m1 = sel.tile([32, 8], dt, tag="m1", name="m1")
            nc.vector.max(out=m1[:, :], in_=nrmT[:, cs])
            nc.vector.match_replace(out=scr[:, cs], in_to_replace=m1[:, :],
                                    in_values=nrmT[:, cs], imm_value=-1e30)
            m2 = sel.tile([32, 8], dt, tag="m2", name="m2")
            nc.vector.max(out=m2[:, :], in_=scr[:, cs])
            nc.vector.tensor_scalar(isg2[:, cs], nrmT[:, cs], m2[:, 7:8], None,
                                    op0=ALU.is_ge)
        mps = ps1.tile([128, 512], dt, tag="t1", name="mps")[:, 0:32]
        nc.tensor.transpose(mps[:, :], isg2[:, :], ident[0:32, 0:32])
        nc.vector.tensor_copy(masks_g[h][:, :], mps[:, :])
        nc.vector.tensor_scalar(masks_l[h][:, :], masks_g[h][:, :],
                                -1.0, 1.0, op0=ALU.mult, op1=ALU.add)

    # ---- V variants ----
    for tt in range(32):
        for h in range(4):
            sl = slice(tt * 260 + h * 65, tt * 260 + (h + 1) * 65)
            nc.vector.tensor_scalar_mul(vl[:, sl], vext[:, sl], masks_l[h][:, tt:tt + 1])
            nc.vector.tensor_scalar_mul(vg[:, sl], vext[:, sl], masks_g[h][:, tt:tt + 1])

    # ---- BOS key scores: ebos rows {0,32} = heads {even,odd} of pair ----
    for p in range(NPAIR):
        for tch in range(8):
            t0 = tch * 512
            bps = ps1.tile([128, 512], dt, tag="t1", name="bps")
            nc.tensor.matmul(bps[0:1, :], kTb[p][0:64, 0:1], qT[p][0:64, t0:t0 + 512],
                             start=True, stop=True)
            nc.tensor.matmul(bps[32:33, :], kTb[p][64:128, 0:1], qT[p][64:128, t0:t0 + 512],
                             start=True, stop=True)
            nc.scalar.activation(ebos[p][:, t0:t0 + 512], bps[0:33, :], AF.Exp, scale=0.125)
        nc.sync.dma_start(v0st[p][0:1, :], vext[0:1, (2 * p) * 65:(2 * p) * 65 + 65])
        nc.sync.dma_start(v0st[p][32:33, :], vext[0:1, (2 * p + 1) * 65:(2 * p + 1) * 65 + 65])

    # ================= Phase D: S^T tiles + ctx =================
    e_sb = {}      # (p, j) -> [128, 768] bf16: cols 0:384 head-even, 384:768 head-odd
    row0_ps = r0p.tile([128, 384], dt, tag="row0ps", name="row0ps")
    out_stage0 = const.tile([128, 256], dt)

    def st_tile(j):
        c_lo = 0 if j > 0 else 1
        c_hi = 3 if j < 31 else 2
        q0 = (j - 1 + c_lo) * 128
        qn = (c_hi - c_lo) * 128
        for p in range(NPAIR):
            eps = epps.tile([128, 1024], dt, tag="eps", name="eps")
            for hh in range(2):
                base = hh * 64
                nc.tensor.matmul(eps[:, hh * 512 + c_lo * 128: hh * 512 + c_lo * 128 + qn],
                                 kTb[p][base:base + 64, j * 128:(j + 1) * 128],
                                 qT[p][base:base + 64, q0:q0 + qn],
                                 start=True, stop=True)
                h = p * 2 + hh
                c3 = (h * 32 + j) * 3
                nc.tensor.matmul(row0_ps[:, c3:c3 + 2],
                                 kTb[p][base:base + 64, j * 128:(j + 1) * 128],
                                 q0s[p][base:base + 64, :], start=True, stop=True)
                nc.tensor.matmul(row0_ps[:, c3 + 2:c3 + 3],
                                 kTlo[p][base:base + 64, j * 128:(j + 1) * 128],
                                 q0s[p][base:base + 64, 0:1], start=True, stop=True)
            et = ep.tile([128, 768], bf, tag="et", name="et")
            src_ap = eps[:, :].rearrange("p (hh q) -> p hh q", hh=2)[:, :, c_lo * 128: c_lo * 128 + qn]
            dst_ap = et[:, :].rearrange("p (hh q) -> p hh q", hh=2)[:, :, c_lo * 128: c_lo * 128 + qn]
            nc.scalar.activation(dst_ap, src_ap, AF.Exp, scale=0.125)
            e_sb[(p, j)] = et

    def ctx_chunk(i):
        cps = ctxp.tile([128, 260], dt, tag="cps", name="cps")
        for h in range(4):
            p, hh = h // 2, h % 2
            sl = slice(h * 65, h * 65 + 65)

            def ets(j, r0_=0, r1_=128, q0_=0, q1_=384):
                return e_sb[(p, j)][r0_:r1_, hh * 384 + q0_: hh * 384 + q1_]

            # (a) global middle
            nc.tensor.matmul(cps[:, sl], ets(i, q0_=128, q1_=256),
                             vg[:, i * 260 + h * 65: i * 260 + (h + 1) * 65],
                             start=True, stop=False)
            # local: tiles i-1, i, i+1
            for j in (i - 1, i, i + 1):
                if j < 0 or j > 31:
                    continue
                nc.tensor.matmul(cps[:, sl],
                                 ets(j, q0_=(i - j + 1) * 128, q1_=(i - j + 2) * 128),
                                 vl[:, j * 260 + h * 65: j * 260 + (h + 1) * 65],
                                 start=False, stop=False)
            # (b) global: k-block 2i-1, q-block 2i
            if i >= 1:
                nc.tensor.matmul(cps[0:64, sl], ets(i - 1, 64, 128, 256, 320),
                                 vg[64:128, (i - 1) * 260 + h * 65: (i - 1) * 260 + (h + 1) * 65],
                                 start=False, stop=False)
            # (c) global: k-block 2i+2, q-block 2i+1
            if i <= 30:
                nc.tensor.matmul(cps[64:128, sl], ets(i + 1, 0, 64, 64, 128),
                                 vg[0:64, (i + 1) * 260 + h * 65: (i + 1) * 260 + (h + 1) * 65],
                                 start=False, stop=False)
            # BOS rank-1
            b = 32 * hh
            nc.tensor.matmul(cps[:, sl], ebos[p][b:b + 1, i * 128:(i + 1) * 128],
                             v0st[p][b:b + 1, :], start=False, stop=True)
        ost = out_stage0 if i == 0 else outp.tile([128, 256], dt, tag="ost", name="ost")
        rr = outp.tile([128, 4], dt, tag="rr", name="rr")
        for h in range(4):
            nc.vector.reciprocal(rr[:, h:h + 1], cps[:, h * 65 + 64: h * 65 + 65])
            nc.scalar.activation(ost[:, h * 64:(h + 1) * 64], cps[:, h * 65: h * 65 + 64],
                                 AF.Copy, scale=rr[:, h:h + 1])
        if i != 0:
            nc.sync.dma_start(O.ap()[i * 128:(i + 1) * 128, :], ost[:, :])

    st_tile(0)
    st_tile(1)
    for j in range(2, 34):
        if j <= 31:
            st_tile(j)
        ctx_chunk(j - 2)

    # ================= Phase E: row 0 =================
    c0ps = ps1.tile([128, 512], dt, tag="t1", name="c0ps")[0:1, 0:260]
    ehi = outp.tile([128, 128], dt, tag="ehi", name="ehi")
    elo = outp.tile([128, 128], dt, tag="elo", name="elo")
    elo2 = outp.tile([128, 128], dt, tag="elo2", name="elo2")
    nc.scalar.activation(ehi[:, :], row0_ps[:, 0::3], AF.Exp, scale=1.0)
    nc.scalar.activation(elo[:, :], row0_ps[:, 1::3], AF.Exp, scale=1.0)
    nc.scalar.activation(elo2[:, :], row0_ps[:, 2::3], AF.Exp, scale=1.0)
    nc.vector.tensor_tensor(ehi[:, :], ehi[:, :], elo[:, :], op=ALU.mult)
    nc.vector.tensor_tensor(e0all[:, :], ehi[:, :], elo2[:, :], op=ALU.mult)
    for h in range(4):
        for j in range(32):
            nc.tensor.matmul(c0ps[0:1, h * 65:(h + 1) * 65], e0all[:, h * 32 + j: h * 32 + j + 1],
                             vext[:, j * 260 + h * 65: j * 260 + (h + 1) * 65],
                             start=(j == 0), stop=(j == 31))
    for h in range(4):
        nc.vector.reciprocal(r0[0:1, h:h + 1], c0ps[0:1, h * 65 + 64: h * 65 + 65])
        nc.scalar.activation(row0_sb[0:1, h * 64:(h + 1) * 64],
                             c0ps[0:1, h * 65: h * 65 + 64], AF.Copy,
                             scale=r0[0:1, h:h + 1])
    nc.vector.tensor_copy(out_stage0[0:1, :], row0_sb[0:1, :])
    nc.sync.dma_start(O.ap()[0:128, :], out_stage0[:, :])


def build_program():
    import concourse.bacc as bacc
    import concourse.mybir as mybir
    from concourse.tile import TileContext
    dt = mybir.dt.float32
    nc = bacc.Bacc("TRN2", debug=False)
    bf = mybir.dt.bfloat16
    hidT = nc.dram_tensor("hidT", [HID, T], dt, kind="ExternalInput")
    hidTb = nc.dram_tensor("hidTb", [HID, T], bf, kind="ExternalInput")
    wqT = nc.dram_tensor("wqT", [HID, FPC], bf, kind="ExternalInput")
    wkT = nc.dram_tensor("wkT", [HID, FPC], dt, kind="ExternalInput")
    wkTb = nc.dram_tensor("wkTb", [HID, FPC], bf, kind="ExternalInput")
    wvT = nc.dram_tensor("wvT", [HID, FPC], bf, kind="ExternalInput")
    bias = nc.dram_tensor("bias", [128, 6], dt, kind="ExternalInput")
    bvrow = nc.dram_tensor("bvrow", [1, FPC], bf, kind="ExternalInput")
    bkqrow = nc.dram_tensor("bkqrow", [1, FPC], dt, kind="ExternalInput")
    O = nc.dram_tensor("O", [T, FPC], dt, kind="ExternalOutput")
    from contextlib import ExitStack
    with TileContext(nc) as tc, ExitStack() as ctx:
        emit(nc, tc, ctx, hidT, hidTb, wqT, wkT, wkTb, wvT, bias, bvrow, bkqrow, O)
    nc.compile()
    return nc


def kernel(hidden_states, attention_mask, Wq, bq, Wk, bk, Wv, bv, _profile=None):
    from concourse import bass_utils
    if _CACHE["nc"] is None:
        _CACHE["nc"] = build_program()
    nc = _CACHE["nc"]

    hs = np.ascontiguousarray(np.asarray(hidden_states, np.float32))
    hidTs = [np.ascontiguousarray(hs[n].T) for n in range(N)]
    in_maps = []
    for core in range(8):
        n, hg = core // 4, core % 4
        f0 = hg * FPC
        bq_s = np.ascontiguousarray(np.asarray(bq, np.float32)[f0:f0 + FPC].reshape(2, 128).T)
        bk_s = np.ascontiguousarray(np.asarray(bk, np.float32)[f0:f0 + FPC].reshape(2, 128).T)
        bv_s = np.ascontiguousarray(np.asarray(bv, np.float32)[f0:f0 + FPC].reshape(2, 128).T)
        bias = np.concatenate([bq_s, bk_s, bv_s], axis=1).astype(np.float32)  # [128, 6]
        import ml_dtypes
        bf16 = ml_dtypes.bfloat16
        in_maps.append({
            "hidT": hidTs[n],
            "hidTb": hidTs[n].astype(bf16),
            "wqT": np.ascontiguousarray(np.asarray(Wq, np.float32)[f0:f0 + FPC, :].T).astype(bf16),
            "wkT": np.ascontiguousarray(np.asarray(Wk, np.float32)[f0:f0 + FPC, :].T),
            "wkTb": np.ascontiguousarray(np.asarray(Wk, np.float32)[f0:f0 + FPC, :].T).astype(bf16),
            "wvT": np.ascontiguousarray(np.asarray(Wv, np.float32)[f0:f0 + FPC, :].T).astype(bf16),
            "bias": np.ascontiguousarray(bias),
            "bvrow": np.ascontiguousarray(np.asarray(bv, np.float32)[None, f0:f0 + FPC]).astype(bf16),
            "bkqrow": np.ascontiguousarray((np.asarray(bk, np.float32) + np.asarray(bq, np.float32))[None, f0:f0 + FPC]),
        })
    kwargs = dict(_profile) if _profile else {}
    res = bass_utils.run_bass_kernel_spmd(nc, in_maps, core_ids=list(range(8)), **kwargs)
    out = np.zeros((N, T, HID), np.float32)
    for core in range(8):
        n, hg = core // 4, core % 4
        out[n, :, hg * FPC:(hg + 1) * FPC] = res.results[core]["O"]
    if _profile is not None:
        _CACHE["last_result"] = res
    return out


# revision 22
# speedup vs baseline: 1.1402x; 1.1402x over previous
"""Trainium2 Bass kernel for BlockGlobalSelfAttention (8-core SPMD).

Sharding: core c handles batch n = c//4, head group hg = c%4 (4 heads each).
Each core is fully independent (no collectives).

Algorithm reformulation (validated against the jax reference to ~2e-6):
  - top-16-per-64-block selection via 16th-largest threshold (max8 x2 +
    match_replace), giving 0/1 masks is_g / is_l per token
  - both attention passes share one set of exp(q.k/8) score tiles S^T
    [128 keys x 384 queries] (keys on partitions); selection masks are
    applied multiplicatively on the V side (Vl = V*is_l, Vg = V*is_g,
    with an extra ones*mask column to accumulate denominators)
  - merged output = (ctxU_l + ctxU_g + E_bos*v0) / (Z_l + Z_g + E_bos):
    algebraically identical to the reference's logsumexp-sigmoid merge
  - BOS key (token 0) is an always-attended rank-1 update (matches the
    reference's gk double-count); query row 0 replaced by an unscaled
    full-attention pass
attention_mask is all-zeros per the problem spec and is ignored.
"""
import sys, os
sys.path.insert(0, "/opt/trn_rl_repo")
import numpy as np

N, T, HID, NH, D = 2, 4096, 1024, 16, 64
NB = 64          # 64-token selection blocks per head
NCH = 32         # 128-query chunks
FPC = 256        # features per core (4 heads)
NPAIR = 2        # head pairs per core
W_DT_NP = np.float32

_CACHE = {"nc": None}


def emit(nc, tc, ctx, hidT, hidTb, wqT, wkT, wkTb, wvT, bias, bvrow, bkqrow, O):
    import concourse.mybir as mybir
    from concourse.masks import make_identity
    dt = mybir.dt.float32
    bf = mybir.dt.bfloat16
    AF = mybir.ActivationFunctionType
    ALU = mybir.AluOpType

    const = ctx.enter_context(tc.tile_pool(name="const", bufs=1))
    big = ctx.enter_context(tc.tile_pool(name="big", bufs=1))
    hidp = ctx.enter_context(tc.tile_pool(name="hidp", bufs=9))
    wp = ctx.enter_context(tc.tile_pool(name="wp", bufs=1))
    evp = ctx.enter_context(tc.tile_pool(name="evp", bufs=1))
    ps1 = ctx.enter_context(tc.tile_pool(name="ps1", bufs=2, space="PSUM"))  # shared 2-bank slots
    sel = ctx.enter_context(tc.tile_pool(name="sel", bufs=2))
    ep = ctx.enter_context(tc.tile_pool(name="ep", bufs=8))
    ctxp = ctx.enter_context(tc.tile_pool(name="ctxp", bufs=2, space="PSUM"))
    r0p = ctx.enter_context(tc.tile_pool(name="r0p", bufs=1, space="PSUM"))
    outp = ctx.enter_context(tc.tile_pool(name="outp", bufs=3))

    # ---- constants ----
    bias_sb = const.tile([128, 6], dt)
    nc.sync.dma_start(bias_sb[:, :], bias.ap()[:, :])
    bv_sb = const.tile([1, 256], bf)
    nc.sync.dma_start(bv_sb[:, :], bvrow.ap()[:, :])
    ones_row = const.tile([1, 128], bf)
    nc.vector.memset(ones_row[:, :], 1.0)
    ones_rowf = const.tile([1, 128], dt)
    nc.vector.memset(ones_rowf[:, :], 1.0)
    bkq_sb = const.tile([1, 256], dt)
    nc.sync.dma_start(bkq_sb[:, :], bkqrow.ap()[:, :])
    ident = const.tile([128, 128], dt)
    make_identity(nc, ident[:, :])

    # ---- weights ----
    w_sb = {}
    for name, wt, wdt in (("q", wqT, bf), ("k", wkT, dt), ("kb", wkTb, bf), ("v", wvT, bf)):
        w = wp.tile([128, 8 * 256], wdt, tag=f"w{name}", name=f"w{name}")
        for c in range(8):
            nc.sync.dma_start(
                w[:, c * 256:(c + 1) * 256],
                wt.ap()[c * 128:(c + 1) * 128, :])
        w_sb[name] = w

    # ---- persistent big buffers ----
    qT = [big.tile([128, T], bf, tag=f"qT{p}", name=f"qT{p}") for p in range(NPAIR)]
    kTb = [big.tile([128, T], bf, tag=f"kTb{p}", name=f"kTb{p}") for p in range(NPAIR)]
    kTlo = [big.tile([128, T], bf, tag=f"kTlo{p}", name=f"kTlo{p}") for p in range(NPAIR)]
    nrm_all = big.tile([128, 128], dt, tag="nrm_all", name="nrm_all")
    vext = big.tile([128, 32 * 260], bf, tag="vext", name="vext")
    vl = big.tile([128, 32 * 260], bf, tag="vl", name="vl")
    vg = big.tile([128, 32 * 260], bf, tag="vg", name="vg")
    ebos = [big.tile([33, T], bf, tag=f"ebos{p}", name=f"ebos{p}") for p in range(NPAIR)]
    e0all = big.tile([128, 128], bf, tag="e0all", name="e0all")
    q0f = [big.tile([128, 1], dt, tag=f"q0f{p}", name=f"q0f{p}") for p in range(NPAIR)]
    q0s = [big.tile([128, 2], bf, tag=f"q0s{p}", name=f"q0s{p}") for p in range(NPAIR)]
    zq0 = [big.tile([128, 1], bf, tag=f"zq0{p}", name=f"zq0{p}") for p in range(NPAIR)]
    masks_g = [big.tile([128, 32], dt, tag=f"mg{h}", name=f"mg{h}") for h in range(4)]
    masks_l = [big.tile([128, 32], dt, tag=f"ml{h}", name=f"ml{h}") for h in range(4)]
    v0st = [big.tile([33, 65], bf, tag=f"v0st{p}", name=f"v0st{p}") for p in range(NPAIR)]
    r0 = const.tile([1, 4], dt)
    row0_sb = const.tile([1, 256], dt)

    # ---- PE warmup: dense matmul burst to trip HAM to 2.4 GHz ----
    wu = ps1.tile([128, 1024], dt, tag="t1", name="wu")[:, 0:512]
    wusrc = ident[:, :].rearrange("p (a b) -> p a b", a=1).to_broadcast([128, 4, 128])
    for _ in range(24):
        nc.tensor.matmul(wu[:, :], ident[:, :], wusrc, start=True, stop=True)

    # ================= Phase B: projections =================
    for tch in range(8):
        t0 = tch * 512
        hid = [hidp.tile([128, 512], dt, tag="hid", name=f"hid{tch}_{i}", bufs=9)
               for i in range(8)]
        hidb = [hidp.tile([128, 512], bf, tag="hidb", name=f"hidb{tch}_{i}", bufs=9)
                for i in range(8)]
        for c in range(8):
            nc.sync.dma_start(hid[c][:, :], hidT.ap()[c * 128:(c + 1) * 128, t0:t0 + 512])
            nc.sync.dma_start(hidb[c][:, :], hidTb.ap()[c * 128:(c + 1) * 128, t0:t0 + 512])
        # k^T bf16 (attention); q^T bf16
        for p in range(NPAIR):
            ps = ps1.tile([128, 1024], dt, tag="t1", name="kps")[:, 0:512]
            for c in range(8):
                nc.tensor.matmul(
                    ps[:, :],
                    w_sb["kb"][:, c * 256 + p * 128: c * 256 + (p + 1) * 128],
                    hidb[c][:, :], start=(c == 0), stop=(c == 7))
            nc.scalar.activation(kTb[p][:, t0:t0 + 512], ps[:, :],
                                 AF.Identity, bias=bias_sb[:, 2 + p:3 + p])
            nc.vector.scalar_tensor_tensor(
                kTlo[p][:, t0:t0 + 512], ps[:, :], bias_sb[:, 2 + p:3 + p],
                kTb[p][:, t0:t0 + 512], op0=ALU.add, op1=ALU.subtract)
        for p in range(NPAIR):
            ps = ps1.tile([128, 1024], dt, tag="t1", name="qps")[:, 0:512]
            for c in range(8):
                nc.tensor.matmul(
                    ps[:, :],
                    w_sb["q"][:, c * 256 + p * 128: c * 256 + (p + 1) * 128],
                    hidb[c][:, :], start=(c == 0), stop=(c == 7))
            nc.scalar.activation(qT[p][:, t0:t0 + 512], ps[:, :],
                                 AF.Identity, bias=bias_sb[:, p:p + 1])
            if tch == 0:
                nc.vector.memset(zq0[p][:, :], 0.0)
                nc.vector.tensor_scalar_add(q0f[p][:, :], ps[:, 0:1],
                                            bias_sb[:, p:p + 1])
                nc.vector.tensor_copy(q0s[p][:, 0:1], q0f[p][:, :])
                nc.vector.tensor_tensor(q0s[p][:, 1:2], q0f[p][:, :], q0s[p][:, 0:1],
                                        op=ALU.subtract)
        # V natural (tokens on partitions), strided into [V|1] groups of 65
        for st in range(4):
            tt = tch * 4 + st
            ps = ps1.tile([128, 1024], dt, tag="t1", name="vps")[:, 0:260]
            psv = ps[:, :].rearrange("p (h d) -> p h d", h=4)[:, :, 0:64]
            for c in range(8):
                nc.tensor.matmul(psv, hidb[c][:, st * 128:(st + 1) * 128],
                                 w_sb["v"][:, c * 256:(c + 1) * 256],
                                 start=(c == 0), stop=False)
            nc.tensor.matmul(psv, ones_row[:, 0:128],
                             bv_sb[:, :], start=False, stop=True)
            nc.vector.tensor_copy(vext[:, tt * 260:(tt + 1) * 260], ps[:, :])
            vv = vext[:, tt * 260:(tt + 1) * 260].rearrange("p (h d) -> p h d", h=4)
            nc.vector.memset(vv[:, :, 64:65], 1.0)
        # k natural (fp32, PSUM-only) -> squares -> per-head norms [128 tok, 4]
        for st in range(4):
            tt = tch * 4 + st
            kn = ps1.tile([128, 1024], dt, tag="t1", name="kn")[:, 0:256]
            for c in range(8):
                nc.tensor.matmul(kn[:, :], hid[c][:, st * 128:(st + 1) * 128],
                                 w_sb["k"][:, c * 256:(c + 1) * 256],
                                 start=(c == 0), stop=False)
            nc.tensor.matmul(kn[:, :], ones_rowf[:, 0:128],
                             bkq_sb[:, :], start=False, stop=True)
            sqn = evp.tile([128, 256], dt, tag="sqn", name=f"sqn{tt}", bufs=3)
            nc.scalar.activation(sqn[:, :], kn[:, :], AF.Square)
            nc.vector.tensor_reduce(
                nrm_all[:, tt * 4:(tt + 1) * 4],
                sqn[:, :].rearrange("p (h d) -> p h d", h=4),
                axis=mybir.AxisListType.X, op=ALU.add)

    # ================= Phase C: selection =================
    for h in range(4):
        ntp = ps1.tile([128, 1024], dt, tag="t1", name="ntp")[0:32, 0:128]
        nc.tensor.transpose(ntp[:, :], nrm_all[:, h:128:4], ident[:, :])
        nrmT = sel.tile([32, 128], dt, tag="nrmT")
        nc.vector.tensor_copy(nrmT[:, :], ntp[:, :])
        scr = sel.tile([32, 128], dt, tag="scr")
        isg2 = sel.tile([32, 128], dt, tag="isg2")
        for s in range(2):
            cs = slice(s * 64, s * 64 + 64)
            m1 = sel.tile([32, 8], dt, tag="m1", name="m1")
            nc.vector.max(out=m1[:, :], in_=nrmT[:, cs])
            top8 = sel.tile([32, 128], dt, tag="top8", name="top8")
            nc.vector.tensor_scalar(top8[:, cs], nrmT[:, cs], m1[:, 7:8], None,
                                    op0=ALU.is_ge)
            # scr = nrm - 1e30 * [nrm >= 8th-largest]
            nc.vector.scalar_tensor_tensor(scr[:, cs], top8[:, cs], -1e30,
                                           nrmT[:, cs], op0=ALU.mult, op1=ALU.add)
            m2 = sel.tile([32, 8], dt, tag="m2", name="m2")
            nc.vector.max(out=m2[:, :], in_=scr[:, cs])
            nc.vector.tensor_scalar(isg2[:, cs], nrmT[:, cs], m2[:, 7:8], None,
                                    op0=ALU.is_ge)
        mps = ps1.tile([128, 1024], dt, tag="t1", name="mps")[:, 0:32]
        nc.tensor.transpose(mps[:, :], isg2[:, :], ident[0:32, 0:32])
        nc.vector.tensor_copy(masks_g[h][:, :], mps[:, :])
        nc.vector.tensor_scalar(masks_l[h][:, :], masks_g[h][:, :],
                                -1.0, 1.0, op0=ALU.mult, op1=ALU.add)

    # ---- V variants ----
    for tt in range(32):
        for h in range(4):
            sl = slice(tt * 260 + h * 65, tt * 260 + (h + 1) * 65)
            nc.vector.tensor_scalar_mul(vl[:, sl], vext[:, sl], masks_l[h][:, tt:tt + 1])
            nc.vector.tensor_scalar_mul(vg[:, sl], vext[:, sl], masks_g[h][:, tt:tt + 1])

    # ---- BOS key scores: ebos rows {0,32} = heads {even,odd} of pair ----
    for p in range(NPAIR):
        for tch in range(8):
            t0 = tch * 512
            bps = ps1.tile([128, 512], dt, tag="t1", name="bps")
            nc.tensor.matmul(bps[0:1, :], kTb[p][0:64, 0:1], qT[p][0:64, t0:t0 + 512],
                             start=True, stop=True)
            nc.tensor.matmul(bps[32:33, :], kTb[p][64:128, 0:1], qT[p][64:128, t0:t0 + 512],
                             start=True, stop=True)
            nc.scalar.activation(ebos[p][:, t0:t0 + 512], bps[0:33, :], AF.Exp, scale=0.125)
        nc.sync.dma_start(v0st[p][0:1, :], vext[0:1, (2 * p) * 65:(2 * p) * 65 + 65])
        nc.sync.dma_start(v0st[p][32:33, :], vext[0:1, (2 * p + 1) * 65:(2 * p + 1) * 65 + 65])

    # ================= Phase D: S^T tiles + ctx =================
    e_sb = {}      # (p, j) -> [128, 768] bf16: cols 0:384 head-even, 384:768 head-odd
    row0_ps = r0p.tile([128, 384], dt, tag="row0ps", name="row0ps")
    out_stage0 = const.tile([128, 256], dt)

    def st_tile(j):
        c_lo = 0 if j > 0 else 1
        c_hi = 3 if j < 31 else 2
        q0 = (j - 1 + c_lo) * 128
        qn = (c_hi - c_lo) * 128
        for p in range(NPAIR):
            eps = ps1.tile([128, 1024], dt, tag="t1", name="eps")
            for hh in range(2):
                base = hh * 64
                nc.tensor.matmul(eps[:, hh * 512 + c_lo * 128: hh * 512 + c_lo * 128 + qn],
                                 kTb[p][base:base + 64, j * 128:(j + 1) * 128],
                                 qT[p][base:base + 64, q0:q0 + qn],
                                 start=True, stop=True)
                h = p * 2 + hh
                c3 = (h * 32 + j) * 3
                nc.tensor.matmul(row0_ps[:, c3:c3 + 2],
                                 kTb[p][base:base + 64, j * 128:(j + 1) * 128],
                                 q0s[p][base:base + 64, :], start=True, stop=True)
                nc.tensor.matmul(row0_ps[:, c3 + 2:c3 + 3],
                                 kTlo[p][base:base + 64, j * 128:(j + 1) * 128],
                                 q0s[p][base:base + 64, 0:1], start=True, stop=True)
            et = ep.tile([128, 768], bf, tag="et", name="et")
            src_ap = eps[:, :].rearrange("p (hh q) -> p hh q", hh=2)[:, :, c_lo * 128: c_lo * 128 + qn]
            dst_ap = et[:, :].rearrange("p (hh q) -> p hh q", hh=2)[:, :, c_lo * 128: c_lo * 128 + qn]
            nc.scalar.activation(dst_ap, src_ap, AF.Exp, scale=0.125)
            e_sb[(p, j)] = et

    def ctx_chunk(i):
        cps = ctxp.tile([128, 260], dt, tag="cps", name="cps")
        for h in range(4):
            p, hh = h // 2, h % 2
            sl = slice(h * 65, h * 65 + 65)

            def ets(j, r0_=0, r1_=128, q0_=0, q1_=384):
                return e_sb[(p, j)][r0_:r1_, hh * 384 + q0_: hh * 384 + q1_]

            # (a) global middle
            nc.tensor.matmul(cps[:, sl], ets(i, q0_=128, q1_=256),
                             vg[:, i * 260 + h * 65: i * 260 + (h + 1) * 65],
                             start=True, stop=False)
            # local: tiles i-1, i, i+1
            for j in (i - 1, i, i + 1):
                if j < 0 or j > 31:
                    continue
                nc.tensor.matmul(cps[:, sl],
                                 ets(j, q0_=(i - j + 1) * 128, q1_=(i - j + 2) * 128),
                                 vl[:, j * 260 + h * 65: j * 260 + (h + 1) * 65],
                                 start=False, stop=False)
            # (b) global: k-block 2i-1, q-block 2i
            if i >= 1:
                nc.tensor.matmul(cps[0:64, sl], ets(i - 1, 64, 128, 256, 320),
                                 vg[64:128, (i - 1) * 260 + h * 65: (i - 1) * 260 + (h + 1) * 65],
                                 start=False, stop=False)
            # (c) global: k-block 2i+2, q-block 2i+1
            if i <= 30:
                nc.tensor.matmul(cps[64:128, sl], ets(i + 1, 0, 64, 64, 128),
                                 vg[0:64, (i + 1) * 260 + h * 65: (i + 1) * 260 + (h + 1) * 65],
                                 start=False, stop=False)
            # BOS rank-1
            b = 32 * hh
            nc.tensor.matmul(cps[:, sl], ebos[p][b:b + 1, i * 128:(i + 1) * 128],
                             v0st[p][b:b + 1, :], start=False, stop=True)
        ost = out_stage0 if i == 0 else outp.tile([128, 256], dt, tag="ost", name="ost")
        rr = outp.tile([128, 4], dt, tag="rr", name="rr")
        nc.vector.reciprocal(rr[:, :], cps[:, 64::65])
        for h in range(4):
            if h % 2 == 0:
                nc.scalar.activation(ost[:, h * 64:(h + 1) * 64], cps[:, h * 65: h * 65 + 64],
                                     AF.Copy, scale=rr[:, h:h + 1])
            else:
                nc.vector.tensor_scalar_mul(ost[:, h * 64:(h + 1) * 64],
                                            cps[:, h * 65: h * 65 + 64], rr[:, h:h + 1])
        if i != 0:
            nc.sync.dma_start(O.ap()[i * 128:(i + 1) * 128, :], ost[:, :])

    st_tile(0)
    st_tile(1)
    for j in range(2, 34):
        if j <= 31:
            st_tile(j)
        ctx_chunk(j - 2)

    # ================= Phase E: row 0 =================
    c0ps = ps1.tile([128, 1024], dt, tag="t1", name="c0ps")[0:1, 0:260]
    ehi = outp.tile([128, 128], dt, tag="ehi", name="ehi")
    elo = outp.tile([128, 128], dt, tag="elo", name="elo")
    elo2 = outp.tile([128, 128], dt, tag="elo2", name="elo2")
    nc.scalar.activation(ehi[:, :], row0_ps[:, 0::3], AF.Exp, scale=1.0)
    nc.scalar.activation(elo[:, :], row0_ps[:, 1::3], AF.Exp, scale=1.0)
    nc.scalar.activation(elo2[:, :], row0_ps[:, 2::3], AF.Exp, scale=1.0)
    nc.vector.tensor_tensor(ehi[:, :], ehi[:, :], elo[:, :], op=ALU.mult)
    nc.vector.tensor_tensor(e0all[:, :], ehi[:, :], elo2[:, :], op=ALU.mult)
    for h in range(4):
        for j in range(32):
            nc.tensor.matmul(c0ps[0:1, h * 65:(h + 1) * 65], e0all[:, h * 32 + j: h * 32 + j + 1],
                             vext[:, j * 260 + h * 65: j * 260 + (h + 1) * 65],
                             start=(j == 0), stop=(j == 31))
    for h in range(4):
        nc.vector.reciprocal(r0[0:1, h:h + 1], c0ps[0:1, h * 65 + 64: h * 65 + 65])
        nc.scalar.activation(row0_sb[0:1, h * 64:(h + 1) * 64],
                             c0ps[0:1, h * 65: h * 65 + 64], AF.Copy,
                             scale=r0[0:1, h:h + 1])
    nc.vector.tensor_copy(out_stage0[0:1, :], row0_sb[0:1, :])
    nc.sync.dma_start(O.ap()[0:128, :], out_stage0[:, :])


def build_program():
    import concourse.bacc as bacc
    import concourse.mybir as mybir
    from concourse.tile import TileContext
    dt = mybir.dt.float32
    nc = bacc.Bacc("TRN2", debug=False)
    bf = mybir.dt.bfloat16
    hidT = nc.dram_tensor("hidT", [HID, T], dt, kind="ExternalInput")
    hidTb = nc.dram_tensor("hidTb", [HID, T], bf, kind="ExternalInput")
    wqT = nc.dram_tensor("wqT", [HID, FPC], bf, kind="ExternalInput")
    wkT = nc.dram_tensor("wkT", [HID, FPC], dt, kind="ExternalInput")
    wkTb = nc.dram_tensor("wkTb", [HID, FPC], bf, kind="ExternalInput")
    wvT = nc.dram_tensor("wvT", [HID, FPC], bf, kind="ExternalInput")
    bias = nc.dram_tensor("bias", [128, 6], dt, kind="ExternalInput")
    bvrow = nc.dram_tensor("bvrow", [1, FPC], bf, kind="ExternalInput")
    bkqrow = nc.dram_tensor("bkqrow", [1, FPC], dt, kind="ExternalInput")
    O = nc.dram_tensor("O", [T, FPC], dt, kind="ExternalOutput")
    from contextlib import ExitStack
    with TileContext(nc) as tc, ExitStack() as ctx:
        emit(nc, tc, ctx, hidT, hidTb, wqT, wkT, wkTb, wvT, bias, bvrow, bkqrow, O)
    nc.compile()
    return nc


def kernel(hidden_states, attention_mask, Wq, bq, Wk, bk, Wv, bv, _profile=None):
    from concourse import bass_utils
    if _CACHE["nc"] is None:
        _CACHE["nc"] = build_program()
    nc = _CACHE["nc"]

    hs = np.ascontiguousarray(np.asarray(hidden_states, np.float32))
    hidTs = [np.ascontiguousarray(hs[n].T) for n in range(N)]
    in_maps = []
    for core in range(8):
        n, hg = core // 4, core % 4
        f0 = hg * FPC
        bq_s = np.ascontiguousarray(np.asarray(bq, np.float32)[f0:f0 + FPC].reshape(2, 128).T)
        bk_s = np.ascontiguousarray(np.asarray(bk, np.float32)[f0:f0 + FPC].reshape(2, 128).T)
        bv_s = np.ascontiguousarray(np.asarray(bv, np.float32)[f0:f0 + FPC].reshape(2, 128).T)
        bias = np.concatenate([bq_s, bk_s, bv_s], axis=1).astype(np.float32)  # [128, 6]
        import ml_dtypes
        bf16 = ml_dtypes.bfloat16
        in_maps.append({
            "hidT": hidTs[n],
            "hidTb": hidTs[n].astype(bf16),
            "wqT": np.ascontiguousarray(np.asarray(Wq, np.float32)[f0:f0 + FPC, :].T).astype(bf16),
            "wkT": np.ascontiguousarray(np.asarray(Wk, np.float32)[f0:f0 + FPC, :].T),
            "wkTb": np.ascontiguousarray(np.asarray(Wk, np.float32)[f0:f0 + FPC, :].T).astype(bf16),
            "wvT": np.ascontiguousarray(np.asarray(Wv, np.float32)[f0:f0 + FPC, :].T).astype(bf16),
            "bias": np.ascontiguousarray(bias),
            "bvrow": np.ascontiguousarray(np.asarray(bv, np.float32)[None, f0:f0 + FPC]).astype(bf16),
            "bkqrow": np.ascontiguousarray((np.asarray(bk, np.float32) + np.asarray(bq, np.float32))[None, f0:f0 + FPC]),
        })
    kwargs = dict(_profile) if _profile else {}
    res = bass_utils.run_bass_kernel_spmd(nc, in_maps, core_ids=list(range(8)), **kwargs)
    out = np.zeros((N, T, HID), np.float32)
    for core in range(8):
        n, hg = core // 4, core % 4
        out[n, :, hg * FPC:(hg + 1) * FPC] = res.results[core]["O"]
    if _profile is not None:
        _CACHE["last_result"] = res
    return out
